# revision 8
# baseline (speedup 1.0000x reference)
"""Trainium2 Bass kernel for GQA attention (B=2, S=2048, D=2048, 16 q-heads /
4 kv-heads, HD=128) with per-head QK RMSNorm + RoPE + causal softmax + output
projection.

Sharding: 8 cores = (batch b in {0,1}) x (kv-group g in {0..3}). Each core
computes its batch's 4 q-heads + 1 kv-head and a partial output through the
row-sharded Wo; the host sums the 4 partials per batch.

Attention computes scores TRANSPOSED (ST[k,t] = K_blk-stationary @ Q moving)
so exp(ST) is directly the moving operand of the PV matmul -- no probability
transposes. The softmax denominator comes from an all-ones stationary matmul
accumulated alongside PV (broadcast across partitions for the normalization).
Phase-1 rope math runs in fp16 on DVE 2x paths, RMSNorm stats on GpSimd, and
q/k head transposes ride the DMA xbar instead of the PE.
"""
import numpy as np

import concourse.bass as bass  # noqa: F401
import concourse.mybir as mybir
import concourse.tile as tile
from concourse import bacc
from concourse.bass_utils import run_bass_kernel_spmd

F32 = mybir.dt.float32
F16 = mybir.dt.float16
AF = mybir.ActivationFunctionType
OP = mybir.AluOpType

B, S, D = 2, 2048, 2048
NH, NKV, HD = 16, 4, 128
REP = NH // NKV
EPS = 1e-6
EXPB = -5.0  # exp bias: cancels in softmax, keeps exp() in fp16 range


def build(s=S):
    """Build + compile the per-core SPMD program (identical on all 8 cores)."""
    sc = s // 128          # s-chunks
    kc = D // 128          # contraction chunks
    nsb = sc // 4          # q superblocks (512 wide)
    nc = bacc.Bacc("TRN2", target_bir_lowering=False, debug=False, num_devices=8)

    xby_d = nc.dram_tensor("xby", [sc, 128, kc * 128], F16, kind="ExternalInput")
    wqkv_d = nc.dram_tensor("wqkv", [D, 768], F16, kind="ExternalInput")
    wo_d = nc.dram_tensor("wo", [512, D], F16, kind="ExternalInput")
    ropes_d = nc.dram_tensor("ropes", [sc, 128, 1280], F16, kind="ExternalInput")
    tri_d = nc.dram_tensor("trimask", [128, 128], F16, kind="ExternalInput")
    out_d = nc.dram_tensor("outp", [s, D], F32, kind="ExternalOutput")

    with tile.TileContext(nc) as tc:
        with (
            tc.tile_pool(name="pers", bufs=1) as pers,
        ):
            qT = pers.tile([128, REP, s], F16, tag="qT")
            kT = pers.tile([128, s], F16, tag="kT")
            vv = pers.tile([128, sc, HD], F16, tag="vv")
            aoT = pers.tile([128, REP, s], F16, tag="aoT")
            tri_t = pers.tile([128, 128], F16, tag="trimask")
            wo_t = pers.tile([128, REP, D], F16, tag="wo")
            nc.scalar.dma_start(
                out=wo_t[:], in_=wo_d.rearrange("(e ki) d -> ki e d", ki=128),
            )
            nc.sync.dma_start(out=tri_t[:], in_=tri_d[:, :])
            eps_t = pers.tile([128, 1], F32, tag="eps")
            nc.vector.memset(eps_t[:], EPS)
            expb_t = pers.tile([128, 1], F32, tag="expb")
            nc.vector.memset(expb_t[:], EXPB)
            ones_t = pers.tile([128, 128], F16, tag="ones")
            nc.vector.memset(ones_t[:], 1.0)

            # ---------------- Phase 1: QKV + RMSNorm + RoPE -----------------
            with (
                tc.tile_pool(name="wq", bufs=1) as wq,
                tc.tile_pool(name="xp", bufs=3) as xp,
                tc.tile_pool(name="cp", bufs=3) as cp,
                tc.tile_pool(name="st", bufs=3) as st,
                tc.tile_pool(name="p1q", bufs=2, space="PSUM") as p1q,
                tc.tile_pool(name="p1kv", bufs=2, space="PSUM") as p1kv,
            ):
                wqkv_t = wq.tile([128, kc, 768], F16, tag="wqkv")
                wqkv_r = wqkv_d.rearrange("(dk ki) e -> ki dk e", ki=128)

                for m in range(sc):
                    # prefetch DMAs first so they issue ahead of dependent work
                    xt = xp.tile([128, kc * 128], F16, tag="xt")
                    nc.sync.dma_start(out=xt[:], in_=xby_d[m])
                    cst = cp.tile([128, 1280], F16, tag="cst")
                    nc.sync.dma_start(out=cst[:], in_=ropes_d[m])
                    if m == 0:
                        for k in range(kc):
                            nc.sync.dma_start(out=wqkv_t[:, k], in_=wqkv_r[:, k])
                    cqw = cst[:, 0:512]
                    sqw = cst[:, 512:1024]
                    ckw = cst[:, 1024:1152]
                    skw = cst[:, 1152:1280]

                    xt3 = xt[:].rearrange("p (dk t) -> p dk t", t=128)
                    pq = p1q.tile([128, 512], F32, tag="p1q")
                    pkv = p1kv.tile([128, 512], F32, tag="p1kv")
                    for k in range(kc):
                        nc.tensor.matmul(
                            pq, xt3[:, k], wqkv_t[:, k, 0:512],
                            start=(k == 0), stop=(k == kc - 1),
                        )
                    for k in range(kc):
                        nc.tensor.matmul(
                            pkv[:, 0:256], xt3[:, k], wqkv_t[:, k, 512:768],
                            start=(k == 0), stop=(k == kc - 1),
                        )

                    # ---- evict to fp16 (scalar engine) ----
                    pq16 = st.tile([128, 512], F16, tag="pq16")
                    nc.scalar.copy(out=pq16[:], in_=pq)
                    pk16 = st.tile([128, 128], F16, tag="pk16")
                    nc.scalar.copy(out=pk16[:], in_=pkv[:, 0:128])
                    nc.scalar.copy(out=vv[:, m, :], in_=pkv[:, 128:256])

                    # ---- RMSNorm stats: gpsimd squares + vector reduce ----
                    sq16 = st.tile([128, 512], F16, tag="sq16")
                    nc.gpsimd.tensor_mul(sq16[:], pq16[:], pq16[:])
                    sk16 = st.tile([128, 128], F16, tag="sk16")
                    nc.gpsimd.tensor_mul(sk16[:], pk16[:], pk16[:])
                    ss = st.tile([128, 16], F32, tag="ss")
                    nc.vector.tensor_reduce(
                        out=ss[:, 0:4],
                        in_=sq16[:].rearrange("p (h d) -> p h d", d=128),
                        axis=mybir.AxisListType.X, op=OP.add,
                    )
                    nc.vector.tensor_reduce(
                        out=ss[:, 4:5],
                        in_=sk16[:].rearrange("p (o d) -> p o d", d=128),
                        axis=mybir.AxisListType.X, op=OP.add,
                    )
                    nc.scalar.activation(
                        ss[:, 8:13], ss[:, 0:5], AF.Sqrt,
                        bias=eps_t[:], scale=1.0 / HD,
                    )
                    rs = st.tile([128, 8], F32, tag="rs")
                    nc.vector.reciprocal(rs[:, 0:5], ss[:, 8:13])

                    # ---- RoPE in fp16 (DVE 2x) ----
                    ra = st.tile([128, 512], F16, tag="ra")
                    nc.vector.tensor_mul(ra[:], pq16[:], cqw)
                    rb = st.tile([128, 512], F16, tag="rb")
                    rb3 = rb[:].rearrange("p (h u d) -> p h u d", u=2, d=64)
                    pq4 = pq16[:].rearrange("p (h u d) -> p h u d", u=2, d=64)
                    sq3 = sqw.rearrange("p (h u d) -> p h u d", u=2, d=64)
                    nc.vector.tensor_mul(rb3[:, :, 0], pq4[:, :, 1], sq3[:, :, 0])
                    nc.vector.tensor_mul(rb3[:, :, 1], pq4[:, :, 0], sq3[:, :, 1])
                    qn0 = st.tile([128, 512], F16, tag="qn0")
                    nc.vector.tensor_add(qn0[:], ra[:], rb[:])
                    qn = st.tile([128, 512], F16, tag="qn")
                    for h in range(REP):
                        nc.vector.tensor_scalar_mul(
                            qn[:, h * 128:(h + 1) * 128],
                            qn0[:, h * 128:(h + 1) * 128],
                            rs[:, h:h + 1],
                        )

                    # ---- k head rope (on gpsimd to offload DVE) ----
                    rak = st.tile([128, 128], F16, tag="rak")
                    nc.gpsimd.tensor_mul(rak[:], pk16[:], ckw)
                    rbk = st.tile([128, 128], F16, tag="rbk")
                    nc.gpsimd.tensor_mul(rbk[:, 0:64], pk16[:, 64:128], skw[:, 0:64])
                    nc.gpsimd.tensor_mul(rbk[:, 64:128], pk16[:, 0:64], skw[:, 64:128])
                    kn0 = st.tile([128, 128], F16, tag="kn0")
                    nc.gpsimd.tensor_add(kn0[:], rak[:], rbk[:])
                    kn = st.tile([128, 128], F16, tag="kn")
                    nc.gpsimd.tensor_scalar_mul(kn[:], kn0[:], rs[:, 4:5])

                    # ---- transposes to head-major via DMA xbar ----
                    for h in range(REP):
                        nc.scalar.dma_start_transpose(
                            out=qT[:, h, m * 128:(m + 1) * 128],
                            in_=qn[:, h * 128:(h + 1) * 128],
                        )
                    nc.sync.dma_start_transpose(
                        out=kT[:, m * 128:(m + 1) * 128], in_=kn[:],
                    )

            # ------- Phase 2+3: causal attention + fused out-projection -----
            with (
                tc.tile_pool(name="ptp", bufs=3) as ptp,
                tc.tile_pool(name="rcp", bufs=3) as rcp,
                tc.tile_pool(name="ob", bufs=2) as ob,
                tc.tile_pool(name="psST", bufs=2, space="PSUM") as psST,  # 2x2 banks
                tc.tile_pool(name="psW", bufs=2, space="PSUM") as psW,    # 2 banks
                tc.tile_pool(name="psPO", bufs=2, space="PSUM") as psPO,  # 2 banks
            ):
                pending = []   # outproj (m, n) groups not yet emitted
                otmap = {}

                def emit_po_group():
                    m, n = pending.pop(0)
                    if n == 0:
                        otmap[m] = ob.tile([128, D], F32, tag="ot", name=f"ot{m}")
                    po = psPO.tile([128, 512], F32, tag="psPO")
                    for e in range(REP):
                        nc.tensor.matmul(
                            po[:], aoT[:, e, m * 128:(m + 1) * 128],
                            wo_t[:, e, n * 512:(n + 1) * 512],
                            start=(e == 0), stop=(e == REP - 1),
                        )
                    nc.vector.tensor_copy(
                        out=otmap[m][:, n * 512:(n + 1) * 512], in_=po[:],
                    )
                    if n == 3:
                        nc.sync.dma_start(
                            out=out_d[m * 128:(m + 1) * 128, :], in_=otmap[m][:],
                        )
                        del otmap[m]

                def attention(Q, h):
                    jlast = 4 * Q + 3
                    nch = 2 * (Q + 1)
                    pvp = psW.tile([128, 512], F32, tag="psW")
                    zb = psW.tile([128, 512], F32, tag="psW")
                    pts = {}

                    def emit_pvz(c):
                        ptc = pts[c]
                        for sl in range(2):
                            j = 2 * c + sl
                            jj = j - 4 * Q
                            c0 = jj * 128 if jj > 0 else 0
                            nc.tensor.matmul(
                                pvp[:, c0:], vv[:, j, :], ptc[:, sl, c0:],
                                start=(j == 0), stop=(j == jlast),
                                skip_group_check=True,
                            )
                            nc.tensor.matmul(
                                zb[:, c0:], ones_t[:], ptc[:, sl, c0:],
                                start=(j == 0), stop=(j == jlast),
                                skip_group_check=True,
                            )

                    for c in range(nch):
                        stc = psST.tile([128, 2, 512], F32, tag="psST")
                        for sl in range(2):
                            j = 2 * c + sl
                            jj = j - 4 * Q
                            c0 = jj * 128 if jj > 0 else 0
                            nc.tensor.matmul(
                                stc[:, sl, c0:],
                                kT[:, j * 128:(j + 1) * 128],
                                qT[:, h, Q * 512 + c0:(Q + 1) * 512],
                                start=True, stop=True,
                            )
                        if c > 0:
                            emit_pvz(c - 1)
                        if pending:
                            emit_po_group()
                        ptc = ptp.tile([128, 2, 512], F16, tag="ptc")
                        pts[c] = ptc
                        nc.scalar.activation(ptc[:], stc[:], AF.Exp, bias=expb_t[:])
                        if c >= nch - 2:
                            # causal triangle on the diagonal 128-blocks
                            for sl in range(2):
                                jj = 2 * c + sl - 4 * Q
                                blk = ptc[:, sl, jj * 128:(jj + 1) * 128]
                                nc.vector.tensor_mul(blk, blk, tri_t[:])
                    emit_pvz(nch - 1)

                    rec = rcp.tile([128, 512], F32, tag="rec")
                    nc.vector.reciprocal_approx_fast(out=rec[:], in_=zb[:])
                    nc.vector.tensor_mul(
                        aoT[:, h, Q * 512:(Q + 1) * 512], pvp[:], rec[:],
                    )

                for Q in range(nsb):
                    for h in range(REP):
                        attention(Q, h)
                    for m in range(4 * Q, 4 * Q + 4):
                        for n in range(D // 512):
                            pending.append((m, n))
                while pending:
                    emit_po_group()

    nc.compile()
    return nc


def make_in_maps(x, cos, sin, Wq, Wk, Wv, Wo, q_norm_w, k_norm_w):
    qsc = (q_norm_w / np.sqrt(HD)).astype(np.float32)
    ksc = k_norm_w.astype(np.float32)

    def rope_consts(w):
        cw = (cos * w[None, :]).astype(np.float32)
        sw = np.empty_like(cw)
        sw[:, :64] = -sin[:, :64] * w[None, 64:]
        sw[:, 64:] = sin[:, 64:] * w[None, :64]
        return cw, sw

    cwq, swq = rope_consts(qsc)
    cwk, swk = rope_consts(ksc)
    ropes = np.concatenate(
        [np.tile(cwq, (1, REP)), np.tile(swq, (1, REP)), cwk, swk], axis=1,
    ).astype(np.float16)
    ropes = np.ascontiguousarray(ropes.reshape(S // 128, 128, 1280))
    r = np.arange(128)
    # trimask[k, t] = 1 where t >= k (valid causal), else 0
    tri = (r[None, :] >= r[:, None]).astype(np.float16)

    in_maps = []
    for c in range(8):
        b, g = c // 4, c % 4
        # xby[m, ki, dk, t] = x[b, m*128 + t, dk*128 + ki]
        xby = np.ascontiguousarray(
            x[b].reshape(S // 128, 128, D // 128, 128).transpose(0, 3, 2, 1)
            .reshape(S // 128, 128, D).astype(np.float16)
        )
        wqkv = np.ascontiguousarray(
            np.concatenate(
                [
                    Wq[:, g * 512:(g + 1) * 512],
                    Wk[:, g * 128:(g + 1) * 128],
                    Wv[:, g * 128:(g + 1) * 128],
                ],
                axis=1,
            ).astype(np.float16)
        )
        wo = np.ascontiguousarray(Wo[g * 512:(g + 1) * 512, :].astype(np.float16))
        in_maps.append(
            dict(
                xby=xby, wqkv=wqkv, wo=wo, ropes=ropes, trimask=tri,
            )
        )
    return in_maps


_cached = None


def kernel(x, cos, sin, Wq, Wk, Wv, Wo, q_norm_w, k_norm_w):
    global _cached
    x = np.asarray(x, np.float32)
    cos = np.asarray(cos, np.float32)
    sin = np.asarray(sin, np.float32)
    in_maps = make_in_maps(
        x, cos, sin,
        np.asarray(Wq, np.float32), np.asarray(Wk, np.float32),
        np.asarray(Wv, np.float32), np.asarray(Wo, np.float32),
        np.asarray(q_norm_w, np.float32), np.asarray(k_norm_w, np.float32),
    )
    if _cached is None:
        _cached = build()
    res = run_bass_kernel_spmd(_cached, in_maps, core_ids=list(range(8)))
    out = np.zeros((B, S, D), np.float64)
    for c in range(8):
        out[c // 4] += res.results[c]["outp"].astype(np.float64)
    return out.astype(np.float32)


# revision 9
# speedup vs baseline: 1.3212x; 1.3212x over previous
"""Trainium2 Bass kernel for GQA attention (B=2, S=2048, D=2048, 16 q-heads /
4 kv-heads, HD=128) with per-head QK RMSNorm + RoPE + causal softmax + output
projection.

Sharding: 8 cores = (batch b in {0,1}) x (kv-group g in {0..3}). Each core
computes its batch's 4 q-heads + 1 kv-head and a partial output through the
row-sharded Wo; the host sums the 4 partials per batch.

Attention computes scores TRANSPOSED (ST[k,t] = K_blk-stationary @ Q moving)
so exp(ST) is directly the moving operand of the PV matmul -- no probability
transposes. The softmax denominator comes from an all-ones stationary matmul
accumulated alongside PV (broadcast across partitions for the normalization).
Phase-1 rope math runs in fp16 on DVE 2x paths, RMSNorm stats on GpSimd, and
q/k head transposes ride the DMA xbar instead of the PE.
"""
import numpy as np

import concourse.bass as bass  # noqa: F401
import concourse.mybir as mybir
import concourse.tile as tile
from concourse import bacc
from concourse.bass_utils import run_bass_kernel_spmd

F32 = mybir.dt.float32
F16 = mybir.dt.float16
AF = mybir.ActivationFunctionType
OP = mybir.AluOpType

B, S, D = 2, 2048, 2048
NH, NKV, HD = 16, 4, 128
REP = NH // NKV
EPS = 1e-6
EXPB = -5.0  # exp bias: cancels in softmax, keeps exp() in fp16 range


def build(s=S):
    """Build + compile the per-core SPMD program (identical on all 8 cores)."""
    sc = s // 128          # s-chunks
    kc = D // 128          # contraction chunks
    nsb = sc // 4          # q superblocks (512 wide)
    nc = bacc.Bacc("TRN2", target_bir_lowering=False, debug=False, num_devices=8)

    xby_d = nc.dram_tensor("xby", [sc, 128, kc * 128], F16, kind="ExternalInput")
    wqkv_d = nc.dram_tensor("wqkv", [D, 768], F16, kind="ExternalInput")
    wo_d = nc.dram_tensor("wo", [512, D], F16, kind="ExternalInput")
    ropes_d = nc.dram_tensor("ropes", [sc, 128, 1280], F16, kind="ExternalInput")
    tri_d = nc.dram_tensor("trimask", [128, 128], F16, kind="ExternalInput")
    iden16_d = nc.dram_tensor("ident16", [128, 128], F16, kind="ExternalInput")
    out_d = nc.dram_tensor("outp", [s, D], F32, kind="ExternalOutput")

    with tile.TileContext(nc) as tc:
        with (
            tc.tile_pool(name="pers", bufs=1) as pers,
        ):
            qT = pers.tile([128, REP, s], F16, tag="qT")
            kT = pers.tile([128, s], F16, tag="kT")
            vv = pers.tile([128, sc, HD], F16, tag="vv")
            aoT = pers.tile([128, REP, s], F16, tag="aoT")
            tri_t = pers.tile([128, 128], F16, tag="trimask")
            wo_t = pers.tile([128, REP, D], F16, tag="wo")
            nc.scalar.dma_start(
                out=wo_t[:], in_=wo_d.rearrange("(e ki) d -> ki e d", ki=128),
            )
            nc.sync.dma_start(out=tri_t[:], in_=tri_d[:, :])
            iden16_t = pers.tile([128, 128], F16, tag="ident16")
            nc.sync.dma_start(out=iden16_t[:], in_=iden16_d[:, :])
            eps_t = pers.tile([128, 1], F32, tag="eps")
            nc.vector.memset(eps_t[:], EPS)
            expb_t = pers.tile([128, 1], F32, tag="expb")
            nc.vector.memset(expb_t[:], EXPB)
            ones_t = pers.tile([128, 128], F16, tag="ones")
            nc.vector.memset(ones_t[:], 1.0)

            # ---------------- Phase 1: QKV + RMSNorm + RoPE -----------------
            with (
                tc.tile_pool(name="wq", bufs=1) as wq,
                tc.tile_pool(name="xp", bufs=4) as xp,
                tc.tile_pool(name="cp", bufs=4) as cp,
                tc.tile_pool(name="st", bufs=3) as st,
                tc.tile_pool(name="p1q", bufs=2, space="PSUM") as p1q,
                tc.tile_pool(name="p1kv", bufs=2, space="PSUM") as p1kv,
                tc.tile_pool(name="p1t", bufs=2, space="PSUM") as p1t,
            ):
                wqkv_t = wq.tile([128, kc, 768], F16, tag="wqkv")
                wqkv_r = wqkv_d.rearrange("(dk ki) e -> ki dk e", ki=128)
                prev_qn = None

                def emit_transposes(qn, kn, m):
                    pt = p1t.tile([128, 5, 128], F16, tag="p1t")
                    for h in range(REP):
                        nc.tensor.transpose(
                            pt[:, h], qn[:, h * 128:(h + 1) * 128], iden16_t[:],
                        )
                        nc.vector.tensor_copy(
                            out=qT[:, h, m * 128:(m + 1) * 128], in_=pt[:, h],
                        )
                    nc.tensor.transpose(pt[:, 4], kn[:], iden16_t[:])
                    nc.vector.tensor_copy(
                        out=kT[:, m * 128:(m + 1) * 128], in_=pt[:, 4],
                    )

                for m in range(sc):
                    # prefetch DMAs first so they issue ahead of dependent work
                    xt = xp.tile([128, kc * 128], F16, tag="xt")
                    nc.sync.dma_start(out=xt[:], in_=xby_d[m])
                    cst = cp.tile([128, 1280], F16, tag="cst")
                    nc.sync.dma_start(out=cst[:], in_=ropes_d[m])
                    if m == 0:
                        for k in range(kc):
                            nc.sync.dma_start(out=wqkv_t[:, k], in_=wqkv_r[:, k])
                    cqw = cst[:, 0:512]
                    sqw = cst[:, 512:1024]
                    ckw = cst[:, 1024:1152]
                    skw = cst[:, 1152:1280]

                    xt3 = xt[:].rearrange("p (dk t) -> p dk t", t=128)
                    pq = p1q.tile([128, 512], F32, tag="p1q")
                    pkv = p1kv.tile([128, 512], F32, tag="p1kv")
                    for k in range(kc):
                        nc.tensor.matmul(
                            pq, xt3[:, k], wqkv_t[:, k, 0:512],
                            start=(k == 0), stop=(k == kc - 1),
                        )
                    for k in range(kc):
                        nc.tensor.matmul(
                            pkv[:, 0:256], xt3[:, k], wqkv_t[:, k, 512:768],
                            start=(k == 0), stop=(k == kc - 1),
                        )
                    if prev_qn is not None:
                        emit_transposes(*prev_qn)

                    # ---- evict to fp16 (scalar engine) ----
                    pq16 = st.tile([128, 512], F16, tag="pq16")
                    nc.scalar.copy(out=pq16[:], in_=pq)
                    pk16 = st.tile([128, 128], F16, tag="pk16")
                    nc.scalar.copy(out=pk16[:], in_=pkv[:, 0:128])
                    nc.scalar.copy(out=vv[:, m, :], in_=pkv[:, 128:256])

                    # ---- RMSNorm stats: gpsimd squares + vector reduce ----
                    sq16 = st.tile([128, 512], F16, tag="sq16")
                    nc.gpsimd.tensor_mul(sq16[:], pq16[:], pq16[:])
                    sk16 = st.tile([128, 128], F16, tag="sk16")
                    nc.gpsimd.tensor_mul(sk16[:], pk16[:], pk16[:])
                    ss = st.tile([128, 16], F32, tag="ss")
                    nc.vector.tensor_reduce(
                        out=ss[:, 0:4],
                        in_=sq16[:].rearrange("p (h d) -> p h d", d=128),
                        axis=mybir.AxisListType.X, op=OP.add,
                    )
                    nc.vector.tensor_reduce(
                        out=ss[:, 4:5],
                        in_=sk16[:].rearrange("p (o d) -> p o d", d=128),
                        axis=mybir.AxisListType.X, op=OP.add,
                    )
                    nc.scalar.activation(
                        ss[:, 8:13], ss[:, 0:5], AF.Sqrt,
                        bias=eps_t[:], scale=1.0 / HD,
                    )
                    rs = st.tile([128, 8], F32, tag="rs")
                    nc.vector.reciprocal(rs[:, 0:5], ss[:, 8:13])

                    # ---- RoPE in fp16 (DVE 2x) ----
                    ra = st.tile([128, 512], F16, tag="ra")
                    nc.vector.tensor_mul(ra[:], pq16[:], cqw)
                    rb = st.tile([128, 512], F16, tag="rb")
                    rb3 = rb[:].rearrange("p (h u d) -> p h u d", u=2, d=64)
                    pq4 = pq16[:].rearrange("p (h u d) -> p h u d", u=2, d=64)
                    sq3 = sqw.rearrange("p (h u d) -> p h u d", u=2, d=64)
                    nc.vector.tensor_mul(rb3[:, :, 0], pq4[:, :, 1], sq3[:, :, 0])
                    nc.vector.tensor_mul(rb3[:, :, 1], pq4[:, :, 0], sq3[:, :, 1])
                    qn0 = st.tile([128, 512], F16, tag="qn0")
                    nc.vector.tensor_add(qn0[:], ra[:], rb[:])
                    qn = st.tile([128, 512], F16, tag="qn")
                    for h in range(REP):
                        nc.vector.tensor_scalar_mul(
                            qn[:, h * 128:(h + 1) * 128],
                            qn0[:, h * 128:(h + 1) * 128],
                            rs[:, h:h + 1],
                        )

                    # ---- k head rope ----
                    rak = st.tile([128, 128], F16, tag="rak")
                    nc.vector.tensor_mul(rak[:], pk16[:], ckw)
                    rbk = st.tile([128, 128], F16, tag="rbk")
                    nc.vector.tensor_mul(rbk[:, 0:64], pk16[:, 64:128], skw[:, 0:64])
                    nc.vector.tensor_mul(rbk[:, 64:128], pk16[:, 0:64], skw[:, 64:128])
                    kn0 = st.tile([128, 128], F16, tag="kn0")
                    nc.vector.tensor_add(kn0[:], rak[:], rbk[:])
                    kn = st.tile([128, 128], F16, tag="kn")
                    nc.vector.tensor_scalar_mul(kn[:], kn0[:], rs[:, 4:5])

                    # transposes for this m are deferred into the next
                    # iteration (after its matmuls) so the PE never waits on
                    # the norm/rope chain
                    prev_qn = (qn, kn, m)
                emit_transposes(*prev_qn)

            # ------- Phase 2+3: causal attention + fused out-projection -----
            with (
                tc.tile_pool(name="ptp", bufs=3) as ptp,
                tc.tile_pool(name="rcp", bufs=3) as rcp,
                tc.tile_pool(name="ob", bufs=2) as ob,
                tc.tile_pool(name="psST", bufs=2, space="PSUM") as psST,  # 2x2 banks
                tc.tile_pool(name="psW", bufs=2, space="PSUM") as psW,    # 2 banks
                tc.tile_pool(name="psPO", bufs=2, space="PSUM") as psPO,  # 2 banks
            ):
                pending = []   # outproj (m, n) groups not yet emitted
                otmap = {}

                def emit_po_group():
                    m, n = pending.pop(0)
                    if n == 0:
                        otmap[m] = ob.tile([128, D], F32, tag="ot", name=f"ot{m}")
                    po = psPO.tile([128, 512], F32, tag="psPO")
                    for e in range(REP):
                        nc.tensor.matmul(
                            po[:], aoT[:, e, m * 128:(m + 1) * 128],
                            wo_t[:, e, n * 512:(n + 1) * 512],
                            start=(e == 0), stop=(e == REP - 1),
                        )
                    nc.vector.tensor_copy(
                        out=otmap[m][:, n * 512:(n + 1) * 512], in_=po[:],
                    )
                    if n == 3:
                        nc.sync.dma_start(
                            out=out_d[m * 128:(m + 1) * 128, :], in_=otmap[m][:],
                        )
                        del otmap[m]

                def attention(Q, h):
                    jlast = 4 * Q + 3
                    nch = 2 * (Q + 1)
                    pvp = psW.tile([128, 512], F32, tag="psW")
                    zb = psW.tile([128, 512], F32, tag="psW")
                    pts = {}

                    def emit_pvz(c):
                        ptc = pts[c]
                        for sl in range(2):
                            j = 2 * c + sl
                            jj = j - 4 * Q
                            c0 = jj * 128 if jj > 0 else 0
                            nc.tensor.matmul(
                                pvp[:, c0:], vv[:, j, :], ptc[:, sl, c0:],
                                start=(j == 0), stop=(j == jlast),
                                skip_group_check=True,
                            )
                            nc.tensor.matmul(
                                zb[:, c0:], ones_t[:], ptc[:, sl, c0:],
                                start=(j == 0), stop=(j == jlast),
                                skip_group_check=True,
                            )

                    for c in range(nch):
                        stc = psST.tile([128, 2, 512], F32, tag="psST")
                        for sl in range(2):
                            j = 2 * c + sl
                            jj = j - 4 * Q
                            c0 = jj * 128 if jj > 0 else 0
                            nc.tensor.matmul(
                                stc[:, sl, c0:],
                                kT[:, j * 128:(j + 1) * 128],
                                qT[:, h, Q * 512 + c0:(Q + 1) * 512],
                                start=True, stop=True,
                            )
                        if c > 0:
                            emit_pvz(c - 1)
                        if pending:
                            emit_po_group()
                        ptc = ptp.tile([128, 2, 512], F16, tag="ptc")
                        pts[c] = ptc
                        nc.scalar.activation(ptc[:], stc[:], AF.Exp, bias=expb_t[:])
                        if c >= nch - 2:
                            # causal triangle on the diagonal 128-blocks
                            for sl in range(2):
                                jj = 2 * c + sl - 4 * Q
                                blk = ptc[:, sl, jj * 128:(jj + 1) * 128]
                                nc.vector.tensor_mul(blk, blk, tri_t[:])
                    emit_pvz(nch - 1)

                    rec = rcp.tile([128, 512], F32, tag="rec")
                    nc.vector.reciprocal_approx_fast(out=rec[:], in_=zb[:])
                    nc.vector.tensor_mul(
                        aoT[:, h, Q * 512:(Q + 1) * 512], pvp[:], rec[:],
                    )

                for Q in range(nsb):
                    for h in range(REP):
                        attention(Q, h)
                    for m in range(4 * Q, 4 * Q + 4):
                        for n in range(D // 512):
                            pending.append((m, n))
                while pending:
                    emit_po_group()

    nc.compile()
    return nc


def make_in_maps(x, cos, sin, Wq, Wk, Wv, Wo, q_norm_w, k_norm_w):
    qsc = (q_norm_w / np.sqrt(HD)).astype(np.float32)
    ksc = k_norm_w.astype(np.float32)

    def rope_consts(w):
        cw = (cos * w[None, :]).astype(np.float32)
        sw = np.empty_like(cw)
        sw[:, :64] = -sin[:, :64] * w[None, 64:]
        sw[:, 64:] = sin[:, 64:] * w[None, :64]
        return cw, sw

    cwq, swq = rope_consts(qsc)
    cwk, swk = rope_consts(ksc)
    ropes = np.concatenate(
        [np.tile(cwq, (1, REP)), np.tile(swq, (1, REP)), cwk, swk], axis=1,
    ).astype(np.float16)
    ropes = np.ascontiguousarray(ropes.reshape(S // 128, 128, 1280))
    r = np.arange(128)
    # trimask[k, t] = 1 where t >= k (valid causal), else 0
    tri = (r[None, :] >= r[:, None]).astype(np.float16)
    ident16 = np.eye(128, dtype=np.float16)

    in_maps = []
    for c in range(8):
        b, g = c // 4, c % 4
        # xby[m, ki, dk, t] = x[b, m*128 + t, dk*128 + ki]
        xby = np.ascontiguousarray(
            x[b].reshape(S // 128, 128, D // 128, 128).transpose(0, 3, 2, 1)
            .reshape(S // 128, 128, D).astype(np.float16)
        )
        wqkv = np.ascontiguousarray(
            np.concatenate(
                [
                    Wq[:, g * 512:(g + 1) * 512],
                    Wk[:, g * 128:(g + 1) * 128],
                    Wv[:, g * 128:(g + 1) * 128],
                ],
                axis=1,
            ).astype(np.float16)
        )
        wo = np.ascontiguousarray(Wo[g * 512:(g + 1) * 512, :].astype(np.float16))
        in_maps.append(
            dict(
                xby=xby, wqkv=wqkv, wo=wo, ropes=ropes, trimask=tri,
                ident16=ident16,
            )
        )
    return in_maps


_cached = None


def kernel(x, cos, sin, Wq, Wk, Wv, Wo, q_norm_w, k_norm_w):
    global _cached
    x = np.asarray(x, np.float32)
    cos = np.asarray(cos, np.float32)
    sin = np.asarray(sin, np.float32)
    in_maps = make_in_maps(
        x, cos, sin,
        np.asarray(Wq, np.float32), np.asarray(Wk, np.float32),
        np.asarray(Wv, np.float32), np.asarray(Wo, np.float32),
        np.asarray(q_norm_w, np.float32), np.asarray(k_norm_w, np.float32),
    )
    if _cached is None:
        _cached = build()
    res = run_bass_kernel_spmd(_cached, in_maps, core_ids=list(range(8)))
    out = np.zeros((B, S, D), np.float64)
    for c in range(8):
        out[c // 4] += res.results[c]["outp"].astype(np.float64)
    return out.astype(np.float32)


# revision 10
# speedup vs baseline: 1.3701x; 1.0371x over previous
"""Trainium2 Bass kernel for GQA attention (B=2, S=2048, D=2048, 16 q-heads /
4 kv-heads, HD=128) with per-head QK RMSNorm + RoPE + causal softmax + output
projection.

Sharding: 8 cores = (batch b in {0,1}) x (kv-group g in {0..3}). Each core
computes its batch's 4 q-heads + 1 kv-head and a partial output through the
row-sharded Wo; the host sums the 4 partials per batch.

Attention computes scores TRANSPOSED (ST[k,t] = K_blk-stationary @ Q moving)
so exp(ST) is directly the moving operand of the PV matmul -- no probability
transposes. The softmax denominator comes from an all-ones stationary matmul
accumulated alongside PV (broadcast across partitions for the normalization).
Phase-1 rope math runs in fp16 on DVE 2x paths, RMSNorm stats on GpSimd, and
q/k head transposes ride the DMA xbar instead of the PE.
"""
import numpy as np

import concourse.bass as bass  # noqa: F401
import concourse.mybir as mybir
import concourse.tile as tile
from concourse import bacc
from concourse.bass_utils import run_bass_kernel_spmd

F32 = mybir.dt.float32
F16 = mybir.dt.float16
AF = mybir.ActivationFunctionType
OP = mybir.AluOpType

B, S, D = 2, 2048, 2048
NH, NKV, HD = 16, 4, 128
REP = NH // NKV
EPS = 1e-6
EXPB = -5.0  # exp bias: cancels in softmax, keeps exp() in fp16 range


def build(s=S):
    """Build + compile the per-core SPMD program (identical on all 8 cores)."""
    sc = s // 128          # s-chunks
    kc = D // 128          # contraction chunks
    nsb = sc // 4          # q superblocks (512 wide)
    nc = bacc.Bacc("TRN2", target_bir_lowering=False, debug=False, num_devices=8)

    xby_d = nc.dram_tensor("xby", [sc, 128, kc * 128], F16, kind="ExternalInput")
    wqkv_d = nc.dram_tensor("wqkv", [D, 768], F16, kind="ExternalInput")
    wo_d = nc.dram_tensor("wo", [512, D], F16, kind="ExternalInput")
    ropes_d = nc.dram_tensor("ropes", [sc, 128, 1280], F16, kind="ExternalInput")
    tri_d = nc.dram_tensor("trimask", [128, 128], F16, kind="ExternalInput")
    iden16_d = nc.dram_tensor("ident16", [128, 128], F16, kind="ExternalInput")
    out_d = nc.dram_tensor("outp", [s, D], F32, kind="ExternalOutput")

    with tile.TileContext(nc) as tc:
        with (
            tc.tile_pool(name="pers", bufs=1) as pers,
        ):
            qT = pers.tile([128, REP, s], F16, tag="qT")
            kT = pers.tile([128, s], F16, tag="kT")
            vv = pers.tile([128, sc, HD], F16, tag="vv")
            aoT = pers.tile([128, REP, s], F16, tag="aoT")
            tri_t = pers.tile([128, 128], F16, tag="trimask")
            wo_t = pers.tile([128, REP, D], F16, tag="wo")
            iden16_t = pers.tile([128, 128], F16, tag="ident16")
            eps_t = pers.tile([128, 1], F32, tag="eps")
            nc.vector.memset(eps_t[:], EPS)
            expb_t = pers.tile([128, 1], F32, tag="expb")
            nc.vector.memset(expb_t[:], EXPB)
            ones_t = pers.tile([128, 128], F16, tag="ones")
            nc.vector.memset(ones_t[:], 1.0)

            # ---------------- Phase 1: QKV + RMSNorm + RoPE -----------------
            with (
                tc.tile_pool(name="wq", bufs=1) as wq,
                tc.tile_pool(name="xp", bufs=4) as xp,
                tc.tile_pool(name="cp", bufs=4) as cp,
                tc.tile_pool(name="st", bufs=3) as st,
                tc.tile_pool(name="p1q", bufs=2, space="PSUM") as p1q,
                tc.tile_pool(name="p1kv", bufs=2, space="PSUM") as p1kv,
                tc.tile_pool(name="p1t", bufs=2, space="PSUM") as p1t,
            ):
                wqkv_t = wq.tile([128, kc, 768], F16, tag="wqkv")
                wqkv_r = wqkv_d.rearrange("(dk ki) e -> ki dk e", ki=128)
                prev_qn = None

                def emit_transposes(qn, kn, m):
                    pt = p1t.tile([128, 5, 128], F16, tag="p1t")
                    for h in range(REP):
                        nc.tensor.transpose(
                            pt[:, h], qn[:, h * 128:(h + 1) * 128], iden16_t[:],
                        )
                        if h < 2:
                            nc.vector.tensor_copy(
                                out=qT[:, h, m * 128:(m + 1) * 128], in_=pt[:, h],
                            )
                        else:
                            nc.scalar.copy(
                                out=qT[:, h, m * 128:(m + 1) * 128], in_=pt[:, h],
                            )
                    nc.tensor.transpose(pt[:, 4], kn[:], iden16_t[:])
                    nc.scalar.copy(
                        out=kT[:, m * 128:(m + 1) * 128], in_=pt[:, 4],
                    )

                for m in range(sc):
                    # prefetch DMAs first so they issue ahead of dependent work
                    xt = xp.tile([128, kc * 128], F16, tag="xt")
                    nc.sync.dma_start(out=xt[:], in_=xby_d[m])
                    cst = cp.tile([128, 1280], F16, tag="cst")
                    nc.sync.dma_start(out=cst[:], in_=ropes_d[m])
                    if m == 0:
                        # split the weight/constant preload across both DMA
                        # queues so the first matmuls start ASAP
                        for k in range(kc // 2):
                            nc.sync.dma_start(out=wqkv_t[:, k], in_=wqkv_r[:, k])
                        for k in range(kc // 2, kc):
                            nc.scalar.dma_start(out=wqkv_t[:, k], in_=wqkv_r[:, k])
                        nc.scalar.dma_start(out=tri_t[:], in_=tri_d[:, :])
                        nc.scalar.dma_start(out=iden16_t[:], in_=iden16_d[:, :])
                        nc.scalar.dma_start(
                            out=wo_t[:],
                            in_=wo_d.rearrange("(e ki) d -> ki e d", ki=128),
                        )
                    cqw = cst[:, 0:512]
                    sqw = cst[:, 512:1024]
                    ckw = cst[:, 1024:1152]
                    skw = cst[:, 1152:1280]

                    xt3 = xt[:].rearrange("p (dk t) -> p dk t", t=128)
                    pq = p1q.tile([128, 512], F32, tag="p1q")
                    pkv = p1kv.tile([128, 512], F32, tag="p1kv")
                    for k in range(kc):
                        nc.tensor.matmul(
                            pq, xt3[:, k], wqkv_t[:, k, 0:512],
                            start=(k == 0), stop=(k == kc - 1),
                        )
                    for k in range(kc):
                        nc.tensor.matmul(
                            pkv[:, 0:256], xt3[:, k], wqkv_t[:, k, 512:768],
                            start=(k == 0), stop=(k == kc - 1),
                        )
                    if prev_qn is not None:
                        emit_transposes(*prev_qn)

                    # ---- evict to fp16 (scalar engine) ----
                    pq16 = st.tile([128, 512], F16, tag="pq16")
                    nc.scalar.copy(out=pq16[:], in_=pq)
                    pk16 = st.tile([128, 128], F16, tag="pk16")
                    nc.scalar.copy(out=pk16[:], in_=pkv[:, 0:128])
                    nc.scalar.copy(out=vv[:, m, :], in_=pkv[:, 128:256])

                    # ---- RMSNorm stats: squares + reduce ----
                    sq16 = st.tile([128, 512], F16, tag="sq16")
                    nc.vector.tensor_mul(sq16[:], pq16[:], pq16[:])
                    sk16 = st.tile([128, 128], F16, tag="sk16")
                    nc.vector.tensor_mul(sk16[:], pk16[:], pk16[:])
                    ss = st.tile([128, 16], F32, tag="ss")
                    nc.vector.tensor_reduce(
                        out=ss[:, 0:4],
                        in_=sq16[:].rearrange("p (h d) -> p h d", d=128),
                        axis=mybir.AxisListType.X, op=OP.add,
                    )
                    nc.vector.tensor_reduce(
                        out=ss[:, 4:5],
                        in_=sk16[:].rearrange("p (o d) -> p o d", d=128),
                        axis=mybir.AxisListType.X, op=OP.add,
                    )
                    nc.scalar.activation(
                        ss[:, 8:13], ss[:, 0:5], AF.Sqrt,
                        bias=eps_t[:], scale=1.0 / HD,
                    )
                    rs = st.tile([128, 8], F32, tag="rs")
                    nc.vector.reciprocal(rs[:, 0:5], ss[:, 8:13])

                    # ---- RoPE in fp16 (DVE 2x) ----
                    ra = st.tile([128, 512], F16, tag="ra")
                    nc.vector.tensor_mul(ra[:], pq16[:], cqw)
                    rb = st.tile([128, 512], F16, tag="rb")
                    rb3 = rb[:].rearrange("p (h u d) -> p h u d", u=2, d=64)
                    pq4 = pq16[:].rearrange("p (h u d) -> p h u d", u=2, d=64)
                    sq3 = sqw.rearrange("p (h u d) -> p h u d", u=2, d=64)
                    nc.vector.tensor_mul(rb3[:, :, 0], pq4[:, :, 1], sq3[:, :, 0])
                    nc.vector.tensor_mul(rb3[:, :, 1], pq4[:, :, 0], sq3[:, :, 1])
                    qn0 = st.tile([128, 512], F16, tag="qn0")
                    nc.vector.tensor_add(qn0[:], ra[:], rb[:])
                    qn = st.tile([128, 512], F16, tag="qn")
                    for h in range(REP):
                        nc.vector.tensor_scalar_mul(
                            qn[:, h * 128:(h + 1) * 128],
                            qn0[:, h * 128:(h + 1) * 128],
                            rs[:, h:h + 1],
                        )

                    # ---- k head rope ----
                    rak = st.tile([128, 128], F16, tag="rak")
                    nc.vector.tensor_mul(rak[:], pk16[:], ckw)
                    rbk = st.tile([128, 128], F16, tag="rbk")
                    nc.vector.tensor_mul(rbk[:, 0:64], pk16[:, 64:128], skw[:, 0:64])
                    nc.vector.tensor_mul(rbk[:, 64:128], pk16[:, 0:64], skw[:, 64:128])
                    kn0 = st.tile([128, 128], F16, tag="kn0")
                    nc.vector.tensor_add(kn0[:], rak[:], rbk[:])
                    kn = st.tile([128, 128], F16, tag="kn")
                    nc.vector.tensor_scalar_mul(kn[:], kn0[:], rs[:, 4:5])

                    # transposes for this m are deferred into the next
                    # iteration (after its matmuls) so the PE never waits on
                    # the norm/rope chain
                    prev_qn = (qn, kn, m)
                emit_transposes(*prev_qn)

            # ------- Phase 2+3: causal attention + fused out-projection -----
            with (
                tc.tile_pool(name="ptp", bufs=3) as ptp,
                tc.tile_pool(name="rcp", bufs=3) as rcp,
                tc.tile_pool(name="ob", bufs=2) as ob,
                tc.tile_pool(name="psST", bufs=2, space="PSUM") as psST,  # 2x2 banks
                tc.tile_pool(name="psW", bufs=2, space="PSUM") as psW,    # 2 banks
                tc.tile_pool(name="psPO", bufs=2, space="PSUM") as psPO,  # 2 banks
            ):
                pending = []   # outproj (m, n) groups not yet emitted
                otmap = {}

                def emit_po_group():
                    m, n = pending.pop(0)
                    if n == 0:
                        otmap[m] = ob.tile([128, D], F32, tag="ot", name=f"ot{m}")
                    po = psPO.tile([128, 512], F32, tag="psPO")
                    for e in range(REP):
                        nc.tensor.matmul(
                            po[:], aoT[:, e, m * 128:(m + 1) * 128],
                            wo_t[:, e, n * 512:(n + 1) * 512],
                            start=(e == 0), stop=(e == REP - 1),
                        )
                    nc.vector.tensor_copy(
                        out=otmap[m][:, n * 512:(n + 1) * 512], in_=po[:],
                    )
                    if n == 3:
                        nc.sync.dma_start(
                            out=out_d[m * 128:(m + 1) * 128, :], in_=otmap[m][:],
                        )
                        del otmap[m]

                def attention(Q, h):
                    jlast = 4 * Q + 3
                    nch = 2 * (Q + 1)
                    pvp = psW.tile([128, 512], F32, tag="psW")
                    zb = psW.tile([128, 512], F32, tag="psW")
                    pts = {}

                    def emit_pvz(c):
                        ptc = pts[c]
                        for sl in range(2):
                            j = 2 * c + sl
                            jj = j - 4 * Q
                            c0 = jj * 128 if jj > 0 else 0
                            nc.tensor.matmul(
                                pvp[:, c0:], vv[:, j, :], ptc[:, sl, c0:],
                                start=(j == 0), stop=(j == jlast),
                                skip_group_check=True,
                            )
                            nc.tensor.matmul(
                                zb[:, c0:], ones_t[:], ptc[:, sl, c0:],
                                start=(j == 0), stop=(j == jlast),
                                skip_group_check=True,
                            )

                    for c in range(nch):
                        stc = psST.tile([128, 2, 512], F32, tag="psST")
                        for sl in range(2):
                            j = 2 * c + sl
                            jj = j - 4 * Q
                            c0 = jj * 128 if jj > 0 else 0
                            nc.tensor.matmul(
                                stc[:, sl, c0:],
                                kT[:, j * 128:(j + 1) * 128],
                                qT[:, h, Q * 512 + c0:(Q + 1) * 512],
                                start=True, stop=True,
                            )
                        if c > 0:
                            emit_pvz(c - 1)
                        if pending:
                            emit_po_group()
                        ptc = ptp.tile([128, 2, 512], F16, tag="ptc")
                        pts[c] = ptc
                        nc.scalar.activation(ptc[:], stc[:], AF.Exp, bias=expb_t[:])
                        if c >= nch - 2:
                            # causal triangle on the diagonal 128-blocks
                            for sl in range(2):
                                jj = 2 * c + sl - 4 * Q
                                blk = ptc[:, sl, jj * 128:(jj + 1) * 128]
                                nc.vector.tensor_mul(blk, blk, tri_t[:])
                    emit_pvz(nch - 1)

                    rec = rcp.tile([128, 512], F32, tag="rec")
                    nc.vector.reciprocal_approx_fast(out=rec[:], in_=zb[:])
                    nc.vector.tensor_mul(
                        aoT[:, h, Q * 512:(Q + 1) * 512], pvp[:], rec[:],
                    )

                for Q in range(nsb):
                    for h in range(REP):
                        attention(Q, h)
                    for m in range(4 * Q, 4 * Q + 4):
                        for n in range(D // 512):
                            pending.append((m, n))
                while pending:
                    emit_po_group()

    nc.compile()
    return nc


def make_in_maps(x, cos, sin, Wq, Wk, Wv, Wo, q_norm_w, k_norm_w):
    qsc = (q_norm_w / np.sqrt(HD)).astype(np.float32)
    ksc = k_norm_w.astype(np.float32)

    def rope_consts(w):
        cw = (cos * w[None, :]).astype(np.float32)
        sw = np.empty_like(cw)
        sw[:, :64] = -sin[:, :64] * w[None, 64:]
        sw[:, 64:] = sin[:, 64:] * w[None, :64]
        return cw, sw

    cwq, swq = rope_consts(qsc)
    cwk, swk = rope_consts(ksc)
    ropes = np.concatenate(
        [np.tile(cwq, (1, REP)), np.tile(swq, (1, REP)), cwk, swk], axis=1,
    ).astype(np.float16)
    ropes = np.ascontiguousarray(ropes.reshape(S // 128, 128, 1280))
    r = np.arange(128)
    # trimask[k, t] = 1 where t >= k (valid causal), else 0
    tri = (r[None, :] >= r[:, None]).astype(np.float16)
    ident16 = np.eye(128, dtype=np.float16)

    in_maps = []
    for c in range(8):
        b, g = c // 4, c % 4
        # xby[m, ki, dk, t] = x[b, m*128 + t, dk*128 + ki]
        xby = np.ascontiguousarray(
            x[b].reshape(S // 128, 128, D // 128, 128).transpose(0, 3, 2, 1)
            .reshape(S // 128, 128, D).astype(np.float16)
        )
        wqkv = np.ascontiguousarray(
            np.concatenate(
                [
                    Wq[:, g * 512:(g + 1) * 512],
                    Wk[:, g * 128:(g + 1) * 128],
                    Wv[:, g * 128:(g + 1) * 128],
                ],
                axis=1,
            ).astype(np.float16)
        )
        wo = np.ascontiguousarray(Wo[g * 512:(g + 1) * 512, :].astype(np.float16))
        in_maps.append(
            dict(
                xby=xby, wqkv=wqkv, wo=wo, ropes=ropes, trimask=tri,
                ident16=ident16,
            )
        )
    return in_maps


_cached = None


def kernel(x, cos, sin, Wq, Wk, Wv, Wo, q_norm_w, k_norm_w):
    global _cached
    x = np.asarray(x, np.float32)
    cos = np.asarray(cos, np.float32)
    sin = np.asarray(sin, np.float32)
    in_maps = make_in_maps(
        x, cos, sin,
        np.asarray(Wq, np.float32), np.asarray(Wk, np.float32),
        np.asarray(Wv, np.float32), np.asarray(Wo, np.float32),
        np.asarray(q_norm_w, np.float32), np.asarray(k_norm_w, np.float32),
    )
    if _cached is None:
        _cached = build()
    res = run_bass_kernel_spmd(_cached, in_maps, core_ids=list(range(8)))
    out = np.zeros((B, S, D), np.float64)
    for c in range(8):
        out[c // 4] += res.results[c]["outp"].astype(np.float64)
    return out.astype(np.float32)
